# revision 31
# baseline (speedup 1.0000x reference)
"""Trainium2 Bass kernel for the autoregressive policy head (nn_ADM_6511170421537).

v2: fp8 DoubleRow matmuls for the per-step layers.

Structure (per core, pure data parallelism over 8 cores):
  trunk:  h = relu(x@sW0+b) -> relu(@sW1+b) -> relu(@sW2+b)   [B,256] (bf16)
  steps i=0..7 (sequential in i, batch-parallel), fp8 e4m3 DoubleRow:
      x1 = relu((h8@(16*W_in[i][:256]) + means@(16*Wext) + 16*b_in[i])/16)
           - one DR matmul per batch tile (K=256 in one pass, 2x MACs)
           - means+bias via K=8 bf16 row-band matmuls (const-1 row at 32j+7)
           - evac: single DVE/ACT op (MULT 1/16, MAX 0) -> e4m3
      x2 = relu((x1@(16*W_h[i]))/16 + b_h[i])
           - DR matmuls; evac: ACT activation(Relu, bias, scale=1/16) -> bf16
  head (bf16, 4-way column-banded) + epilogue: as v1.

Accuracy: e4m3 on x1/x2 only (trunk + head bf16) simulates to ~1.0-1.5%
norm-rel vs the 2e-2 gate.

Evacuations are the binding resource (PSUM->SBUF runs at ~1 elem/cycle on
DVE/ACT, no perf modes for fp32 PSUM sources); a greedy build-time load
balancer assigns each evac to the lighter engine (x2 pinned to ACT for the
fused scale+bias).
"""

import os

os.environ.setdefault("MYCRO_LOCAL_CACHE", "1")

import numpy as np
from contextlib import ExitStack

import concourse.bass as bass
import concourse.bacc as bacc
import concourse.mybir as mybir
import concourse.tile as tile
from concourse.bass_utils import run_bass_kernel_spmd

# ---- problem constants (hardcoded; kernel.py must be self-contained) ----
B = 65536
IN_DIM = 64
HID = 256
D = 8
NCORES = 8
BC = B // NCORES          # 8192 rows per core
BT = 512                  # batch tile (one fp32 PSUM bank of free dim)
NT = BC // BT             # 16 tiles per core
GRP = 4                   # tiles per group
NG = NT // GRP            # 4 groups
WAVEG = 4                 # groups per wave (single wave, 4-deep interleave)
LOG_2PI = float(np.log(2.0 * np.pi))
WS = 16.0                 # fp8 weight prescale
INV = 1.0 / WS

F32 = mybir.dt.float32
BF16 = mybir.dt.bfloat16
FP8 = mybir.dt.float8e4
RELU = mybir.ActivationFunctionType.Relu
EXP = mybir.ActivationFunctionType.Exp
ADD = mybir.AluOpType.add
MAX = mybir.AluOpType.max
MIN = mybir.AluOpType.min
MULT = mybir.AluOpType.mult
DR = mybir.MatmulPerfMode.DoubleRow

TRACE = False           # test.py flips this to get the NTFF profile
_NC_CACHE = {}


def _build_bass():
    nc = bacc.Bacc()

    # x in "L0 row-tiling" layout: partition 64h+c holds feature c of tiles
    # j in {2h, 2h+1} of each group; free = (group, j%2, batch)
    xT = nc.declare_dram_parameter("xT", [128, BC // 2], BF16, isOutput=False)
    epsE = nc.declare_dram_parameter("epsE", [128, BT], BF16, isOutput=False)
    # wa: L0 weights (sW0 duplicated in both 64-row halves)
    wa = nc.declare_dram_parameter("wa", [128, HID], BF16, isOutput=False)
    # wt8: [p, tl(0=L1,1=L2), u, mf] = 16*sW[128u+p, mf], e4m3
    wt8 = nc.declare_dram_parameter("wt8", [128, 2 * 2 * HID], FP8, isOutput=False)
    # wtb: trunk bias bands, row 32j+7 = 16*sb{1,2}
    wtb = nc.declare_dram_parameter("wtb", [128, 2 * 2 * 128], BF16, isOutput=False)
    # w8: [p, i, l(0=W_in,1=W_h), u(k-group), mf] = 16*W[i][128u+p, mf], e4m3
    w8 = nc.declare_dram_parameter("w8", [128, D * 2 * 2 * HID], FP8, isOutput=False)
    # wx2: correction weights, band 32j rows r; l=0 (x1): r<i ->
    # 16*W_in[i][256+r], r=7 -> 16*b_in[i]; l=1 (x2): r=7 -> 16*b_h[i]
    wx2 = nc.declare_dram_parameter("wx2", [128, D * 2 * 2 * 128], BF16, isOutput=False)
    wo = nc.declare_dram_parameter("wo", [128, D * 2 * 2], BF16, isOutput=False)
    bb = nc.declare_dram_parameter("bb", [128, 30], F32, isOutput=False)
    # mg init pattern: zeros with 1.0 at the const-bias rows 32j+7
    mgi = nc.declare_dram_parameter("mgi", [128, BT], BF16, isOutput=False)
    omT = nc.declare_dram_parameter("omT", [D, BC], F32, isOutput=True)
    osT = nc.declare_dram_parameter("osT", [D, BC], F32, isOutput=True)
    olT = nc.declare_dram_parameter("olT", [D, BC], F32, isOutput=True)

    # build-time greedy engine balancer for PSUM evacuations
    eload = {"act": 0.0, "dve": 0.0}

    def _cost(fd):
        # constants fit from measured per-instr times (ACT 1078ns, DVE
        # 1173ns at FD=1024)
        return {"act": (fd + 270.0) / 1.2, "dve": (fd + 100.0) / 0.96}

    def pick(fd, allowed=("act", "dve")):
        cost = _cost(fd)
        e = min(allowed, key=lambda k: eload[k] + cost[k])
        eload[e] += cost[e]
        return e

    def pick2(fd):
        """Both engines, lighter one first — a unit's two evacs run in
        parallel on ACT and DVE so their latency stays under the PE cover."""
        cost = _cost(fd)
        first = min(("act", "dve"), key=lambda k: eload[k])
        other = "dve" if first == "act" else "act"
        eload[first] += cost[first]
        eload[other] += cost[other]
        return first, other

    with tile.TileContext(nc) as tc, ExitStack() as ctx:
        wp = ctx.enter_context(tc.tile_pool(name="w", bufs=1))
        hpool = ctx.enter_context(tc.tile_pool(name="h8", bufs=NG))
        mgpool = ctx.enter_context(tc.tile_pool(name="mg", bufs=NG))
        tpool = ctx.enter_context(tc.tile_pool(name="tr", bufs=4))
        x1pool = ctx.enter_context(tc.tile_pool(name="x1", bufs=4))
        x2pool = ctx.enter_context(tc.tile_pool(name="x2", bufs=4))
        smpool = ctx.enter_context(tc.tile_pool(name="sm", bufs=8))
        opool = ctx.enter_context(tc.tile_pool(name="out", bufs=1))
        pspair = ctx.enter_context(tc.tile_pool(name="pspair", bufs=4, space="PSUM"))

        # ---- staged loads over the 3 DMA queues; critical path first
        xts_s = wp.tile([128, NG, 2, BT], BF16)
        xv = xT[:].rearrange("p (g u b) -> p g u b", g=NG, u=2)
        wa_s = wp.tile([128, HID], BF16)
        wt8_s = wp.tile([128, 2, 2, HID], FP8)
        wtb_s = wp.tile([128, 2, 2, 128], BF16)
        bb_s = wp.tile([128, 30], F32)
        w8_s = wp.tile([128, D, 2, 2, HID], FP8)
        wx2_s = wp.tile([128, D, 2, 2, 128], BF16)
        wo_s = wp.tile([128, D, 2, 2], BF16)
        eps_s = wp.tile([128, BT], BF16)

        # ---- PE warmup: dummy matmuls while the input DMAs stream
        wu = wp.tile([128, BT], BF16)
        nc.gpsimd.memset(wu[:], 0.0)
        psw = pspair.tile([128, 2, BT], F32, tag="pspair", name="warm")
        for r in range(9):
            nc.tensor.matmul(psw[:, r % 2, :], wu[0:128, 0:128], wu[:],
                             start=True, stop=True)

        # sync queue: x for groups 0-1 halves, w0, trunk DR weights
        nc.sync.dma_start(xts_s[0:64, 0, :, :], xv[0:64, 0, :, :])
        nc.sync.dma_start(wa_s[:], wa[:])
        nc.sync.dma_start(xts_s[64:128, 0, :, :], xv[64:128, 0, :, :])
        nc.sync.dma_start(wt8_s[:], wt8[:].rearrange("p (l u m) -> p l u m", l=2, u=2))
        nc.sync.dma_start(wtb_s[:], wtb[:].rearrange("p (l m c) -> p l m c", l=2, m=2))
        # scalar queue: wave tiles, biases, step weights
        nc.scalar.dma_start(xts_s[:, 1, :, :], xv[:, 1, :, :])
        nc.scalar.dma_start(bb_s[:], bb[:])
        nc.scalar.dma_start(w8_s[:], w8[:].rearrange("p (i l u m) -> p i l u m",
                                                     i=D, l=2, u=2))
        # gpsimd queue: remaining inputs, mg init, corrections, heads, eps
        nc.gpsimd.dma_start(xts_s[:, 2, :, :], xv[:, 2, :, :])
        nc.gpsimd.dma_start(xts_s[:, 3, :, :], xv[:, 3, :, :])
        nc.gpsimd.dma_start(wx2_s[:], wx2[:].rearrange("p (i l m c) -> p i l m c",
                                                       i=D, l=2, m=2))
        nc.gpsimd.dma_start(wo_s[:], wo[:].rearrange("p (i k c) -> p i k c",
                                                     i=D, k=2))
        nc.gpsimd.dma_start(eps_s[:], epsE[:])

        b0_s = bb_s[:, 0:2]
        bh_s = bb_s[:, 6:22].rearrange("p (i m) -> p i m", i=D)
        bo_s = bb_s[:, 22:30]

        def evac_relu_bias(dst, src, bias, eng):
            """relu(src + bias)."""
            if eng == "act":
                nc.scalar.activation(dst, src, RELU, bias=bias)
            else:
                nc.vector.tensor_scalar(dst, src, bias, 0.0, ADD, MAX)

        def evac_relu_scale(dst, src, eng):
            """relu(src/16) (bias already accumulated in PSUM)."""
            if eng == "act":
                nc.scalar.activation(dst, src, RELU, scale=INV)
            else:
                nc.vector.tensor_scalar(dst, src, INV, 0.0, MULT, MAX)

        # Activation tiles are per-group: [128, u(2), j(4), BT], feature
        # 128u + p; psum pair tiles [128, s(2), BT] hold slots j = 2p+s.

        def emit_dr_trunk(g, tl, rhs, dst, mg, tag):
            """fp8 DR 256->256 trunk layer over one group; bias via the K=8
            band matmuls against the const-1 row of mg (means still zero)."""
            for m in range(2):
                pss = [pspair.tile([128, 2, BT], F32, tag="pspair",
                                   name=f"ps{tag}{m}{p}") for p in range(2)]
                wv = wt8_s[:, tl, :, bass.ts(m, 128)]
                for p in range(2):
                    for s in range(2):
                        nc.tensor.matmul(
                            pss[p][:, s, :], wv, rhs[:, :, 2 * p + s, :],
                            start=True, stop=False, perf_mode=DR,
                        )
                for j in range(GRP):
                    nc.tensor.matmul(
                        pss[j // 2][:, j % 2, :],
                        wtb_s[32 * j : 32 * j + 8, tl, m, :],
                        mg[32 * j : 32 * j + 8, :],
                        start=False, stop=True,
                        tile_position=(32 * j, 0),
                    )
                engs = pick2(1024)
                for p in range(2):
                    evac_relu_scale(dst[:, m, 2 * p : 2 * p + 2, :],
                                    pss[p][:], engs[p])

        def trunk_wave(groups, states):
            """Trunk for all groups, interleaved at layer granularity."""
            hp = {g: tpool.tile([128, 2, 4, BT], FP8, tag="hp", name=f"hp{g}")
                  for g in groups}
            for g in groups:
                # L0 is K=64 bf16: tile pairs (0,2) and (1,3) run concurrently
                # in the two 64-row halves of the PE array (w0 duplicated).
                for m in range(2):
                    pss = [pspair.tile([128, 2, BT], F32, tag="pspair",
                                       name=f"pst0g{g}{m}{p}") for p in range(2)]
                    for j in (0, 2, 1, 3):
                        h = j // 2
                        nc.tensor.matmul(
                            pss[j // 2][:, j % 2, :],
                            wa_s[64 * h : 64 * h + IN_DIM, bass.ts(m, 128)],
                            xts_s[64 * h : 64 * h + IN_DIM, g, j % 2, :],
                            start=True, stop=True,
                            tile_position=(64 * h, 0),
                        )
                    engs = pick2(1024)
                    for p in range(2):
                        evac_relu_bias(hp[g][:, m, 2 * p : 2 * p + 2, :],
                                       pss[p][:], b0_s[:, m : m + 1], engs[p])
            hq = {g: tpool.tile([128, 2, 4, BT], FP8, tag="hq", name=f"hq{g}")
                  for g in groups}
            for g in groups:
                emit_dr_trunk(g, 0, hp[g], hq[g], states[g]["mg"], f"t1g{g}")
            for g in groups:
                # L2 evac writes the e4m3 h8 tile (consumed by the x1 DRs)
                emit_dr_trunk(g, 1, hq[g], states[g]["h"], states[g]["mg"],
                              f"t2g{g}")

        def step_x1(i, g, h8, mg):
            """x1 = relu((h@16W + means@16Wext + 16b)/16) via DR + K=8 bands."""
            x1t = x1pool.tile([128, 2, 4, BT], FP8, tag="x1", name=f"x1{g}")
            for m in range(2):
                pss = [pspair.tile([128, 2, BT], F32, tag="pspair",
                                   name=f"psl{i}g{g}{m}{p}") for p in range(2)]
                wv = w8_s[:, i, 0, :, bass.ts(m, 128)]
                for p in range(2):
                    for s in range(2):
                        nc.tensor.matmul(
                            pss[p][:, s, :], wv, h8[:, :, 2 * p + s, :],
                            start=True, stop=False, perf_mode=DR,
                        )
                for j in range(GRP):
                    nc.tensor.matmul(
                        pss[j // 2][:, j % 2, :],
                        wx2_s[32 * j : 32 * j + 8, i, 0, m, :],
                        mg[32 * j : 32 * j + 8, :],
                        start=False, stop=True,
                        tile_position=(32 * j, 0),
                    )
                engs = pick2(1024)
                for p in range(2):
                    evac_relu_scale(x1t[:, m, 2 * p : 2 * p + 2, :],
                                    pss[p][:], engs[p])
            return x1t

        def step_x2(i, g, x1t, mg):
            """x2 = relu((x1@16W + 16b)/16) via DR + K=8 bias band (bf16)."""
            x2t = x2pool.tile([128, 2, 4, BT], BF16, tag="x2", name=f"x2{g}")
            for m in range(2):
                pss = [pspair.tile([128, 2, BT], F32, tag="pspair",
                                   name=f"psh{i}g{g}{m}{p}") for p in range(2)]
                wv = w8_s[:, i, 1, :, bass.ts(m, 128)]
                for p in range(2):
                    for s in range(2):
                        nc.tensor.matmul(
                            pss[p][:, s, :], wv, x1t[:, :, 2 * p + s, :],
                            start=True, stop=False, perf_mode=DR,
                        )
                for j in range(GRP):
                    nc.tensor.matmul(
                        pss[j // 2][:, j % 2, :],
                        wx2_s[32 * j : 32 * j + 8, i, 1, m, :],
                        mg[32 * j : 32 * j + 8, :],
                        start=False, stop=True,
                        tile_position=(32 * j, 0),
                    )
                engs = pick2(1024)
                for p in range(2):
                    evac_relu_scale(x2t[:, m, 2 * p : 2 * p + 2, :],
                                    pss[p][:], engs[p])
            return x2t

        def step_head(i, g, x2t, mg, epi):
            """head: 4 tiles' M=2 matmuls concurrent in PE column groups."""
            pst = pspair.tile([128, 2, BT], F32, tag="pspair", name=f"pshd{i}g{g}")
            pso = pst[:, 0, :]
            for k in range(2):
                for j in range(GRP):
                    nc.tensor.matmul(
                        pso[32 * j : 32 * j + 2, :],
                        wo_s[:, i, k, :],
                        x2t[:, k, j, :],
                        start=(k == 0), stop=(k == 1),
                        tile_position=(0, 32 * j),
                    )
            sm = smpool.tile([128, BT], BF16, tag="sm", name=f"sm{g}_{i}")
            evac_relu_bias(sm[0:98, :], pso[0:98, :], bo_s[0:98, i : i + 1],
                           pick(512))
            # scatter: mean_j -> band row 32j+i of mg (correction input for
            # steps i+1..; skipped at i=7) and mean/ls -> the epilogue tile
            # at partition 8t + i.
            smv = sm[:].rearrange("(j r) b -> j r b", j=4)
            mgv = mg[:].rearrange("(j q r) b -> j q r b", j=4, q=4)
            if i < D - 1:
                nc.sync.dma_start(mgv[:, 0, i, :], smv[:, 0, :])
            nc.gpsimd.dma_start(epi[32 * g + i : 32 * g + 32 : 8, 0, :],
                                smv[:, 0, :])
            nc.gpsimd.dma_start(epi[32 * g + i : 32 * g + 32 : 8, 1, :],
                                smv[:, 1, :])

        state = {}
        for g in range(NG):
            state[g] = dict(
                h=hpool.tile([128, 2, 4, BT], FP8, tag="h8", name=f"h8_{g}"),
                mg=mgpool.tile([128, BT], BF16, tag="mg", name=f"mg{g}"),
            )
        # correction bands: zero the mean rows (the K=8 stationary reads rows
        # i..6 before they're written) and set the const-1 bias row 32j+7;
        # the trunk's bias bands also read these, so load them first (sync q)
        for g in range(NG):
            nc.sync.dma_start(state[g]["mg"][:], mgi[:])

        # epilogue tile: [p = 8*tile + step, (mean, log_std), batch]
        epi = opool.tile([128, 2, BT], BF16, tag="epi")
        lsc = opool.tile([128, BT], BF16, tag="lsc")
        mean_f = opool.tile([128, BT], F32, tag="mean_f")
        st = opool.tile([128, BT], BF16, tag="st")
        se = opool.tile([128, BT], BF16, tag="se")
        smp = opool.tile([128, BT], F32, tag="smp")
        sq2 = opool.tile([128, BT], BF16, tag="sq2")
        lp = opool.tile([128, BT], F32, tag="lp")

        # -0.5*eps^2 - 0.5*log(2pi): precomputed while the trunk runs
        nc.gpsimd.tensor_mul(sq2[:], eps_s[:], eps_s[:])
        nc.gpsimd.tensor_scalar(sq2[:], sq2[:], -0.5, -0.5 * LOG_2PI, MULT, ADD)

        omV = omT[:].rearrange("i (t b) -> t i b", t=NT)
        osV = osT[:].rearrange("i (t b) -> t i b", t=NT)
        olV = olT[:].rearrange("i (t b) -> t i b", t=NT)

        def emit_epi_half(hf):
            """Epilogue for partitions [64*hf, 64*hf+64) (tiles 8*hf..)."""
            sub = slice(64 * hf, 64 * hf + 64)
            tl = slice(8 * hf, 8 * hf + 8)
            nc.vector.tensor_single_scalar(lsc[sub, :], epi[sub, 1, :], 2.0, MIN)
            nc.vector.tensor_copy(mean_f[sub, :], epi[sub, 0, :])
            nc.sync.dma_start(omV[tl], mean_f[sub, :])
            nc.scalar.activation(st[sub, :], lsc[sub, :], EXP)
            nc.vector.tensor_mul(se[sub, :], st[sub, :], eps_s[sub, :])
            nc.vector.tensor_add(smp[sub, :], se[sub, :], mean_f[sub, :])
            nc.scalar.dma_start(osV[tl], smp[sub, :])
            nc.vector.tensor_sub(lp[sub, :], sq2[sub, :], lsc[sub, :])
            nc.gpsimd.dma_start(olV[tl], lp[sub, :])

        groups = list(range(NG))
        trunk_wave(groups, state)
        for i in range(D):
            x1s = {g: step_x1(i, g, state[g]["h"], state[g]["mg"])
                   for g in groups}
            x2s = {g: step_x2(i, g, x1s[g], state[g]["mg"]) for g in groups}
            for g in groups:
                step_head(i, g, x2s[g], state[g]["mg"], epi)
                # tiles 0-7 (groups 0,1) complete first: overlap their
                # epilogue under the last groups' heads
                if i == D - 1 and g == 1:
                    emit_epi_half(0)
        emit_epi_half(1)

    nc.compile()
    return nc


def _get_nc():
    if "nc" not in _NC_CACHE:
        _NC_CACHE["nc"] = _build_bass()
    return _NC_CACHE["nc"]


def kernel(**inputs):
    import ml_dtypes

    bf16 = ml_dtypes.bfloat16
    e4m3 = ml_dtypes.float8_e4m3
    inp = {k: np.ascontiguousarray(np.asarray(v, dtype=np.float32))
           for k, v in inputs.items()}
    x = inp["inputs"]
    eps = inp["eps"]
    W_in, b_in = inp["W_in"], inp["b_in"]
    W_h, b_h = inp["W_h"], inp["b_h"]
    W_out, b_out = inp["W_out"], inp["b_out"]

    def cb(a):
        return np.ascontiguousarray(a.astype(bf16))

    def c8(a):
        return np.ascontiguousarray(np.clip(a, -240.0, 240.0).astype(e4m3))

    c = np.ascontiguousarray

    # w8: [p, i, l, u, mf] = 16*W[i][128u+p, mf]
    wi = (W_in[:, :HID, :] * WS).reshape(D, 2, 128, HID).transpose(2, 0, 1, 3)
    wh = (W_h * WS).reshape(D, 2, 128, HID).transpose(2, 0, 1, 3)
    w8_np = np.stack([wi, wh], axis=2)  # [128, D, l, u, HID]

    # wx2 correction bands: [32j + r, i, l, m, c]; l=0: r<i ->
    # 16*W_in[i][256+r], r=7 -> 16*b_in[i]; l=1: r=7 -> 16*b_h[i]
    base = np.zeros((32, D, 2, 2, 128), np.float32)
    for i in range(D):
        if i > 0:
            base[:i, i, 0] = (W_in[i, HID : HID + i, :] * WS).reshape(i, 2, 128)
        base[7, i, 0] = (b_in[i] * WS).reshape(2, 128)
        base[7, i, 1] = (b_h[i] * WS).reshape(2, 128)
    wx2_np = np.tile(base, (4, 1, 1, 1, 1))

    bo_band = np.zeros((128, D), np.float32)
    for j in range(4):
        for ch in range(2):
            bo_band[32 * j + ch, :] = b_out[:, ch]

    wa_np = np.zeros((128, HID), np.float32)
    wa_np[:IN_DIM] = inp["sW0"]
    wa_np[64 : 64 + IN_DIM] = inp["sW0"]  # L0 row-tiling duplicate

    # trunk DR weights (x16, e4m3) and bias bands
    wt8_np = np.stack([
        (inp["sW1"] * WS).reshape(2, 128, HID).transpose(1, 0, 2),
        (inp["sW2"] * WS).reshape(2, 128, HID).transpose(1, 0, 2),
    ], axis=1)  # [128, tl, u, HID]
    wtb_np = np.zeros((32, 2, 2, 128), np.float32)
    wtb_np[7, 0] = (inp["sb1"] * WS).reshape(2, 128)
    wtb_np[7, 1] = (inp["sb2"] * WS).reshape(2, 128)
    wtb_np = np.tile(wtb_np, (4, 1, 1, 1))

    wo_np = W_out.reshape(D, 2, 128, 2).transpose(2, 0, 1, 3)

    bb_np = np.concatenate([
        inp["sb0"].reshape(2, 128).T, inp["sb1"].reshape(2, 128).T,
        inp["sb2"].reshape(2, 128).T,
        b_h.reshape(D, 2, 128).transpose(2, 0, 1).reshape(128, -1),
        bo_band,
    ], axis=1)

    mgi_np = np.zeros((128, BT), np.float32)
    for j in range(4):
        mgi_np[32 * j + 7, :] = 1.0

    shared = {
        "wa": cb(wa_np),
        "wt8": c8(wt8_np).reshape(128, -1),
        "wtb": cb(wtb_np.reshape(128, -1)),
        "w8": c8(w8_np).reshape(128, -1),
        "wx2": cb(wx2_np.reshape(128, -1)),
        "wo": cb(wo_np.reshape(128, -1)),
        "bb": c(bb_np),
        "mgi": cb(mgi_np),
    }

    in_maps = []
    for core in range(NCORES):
        sl = slice(core * BC, (core + 1) * BC)
        m = dict(shared)
        xg = x[sl].T.reshape(IN_DIM, 4, 2, 2, BT).transpose(2, 0, 1, 3, 4)
        m["xT"] = cb(xg.reshape(128, BC // 2))
        m["epsE"] = cb(eps[sl].T.reshape(D, NT, BT).transpose(1, 0, 2).reshape(128, BT))
        in_maps.append(m)

    nc = _get_nc()
    kw = {}
    if TRACE:
        import shutil

        shutil.rmtree("/tmp/ktrace", ignore_errors=True)
        os.makedirs("/tmp/ktrace", exist_ok=True)
        kw = dict(trace=True, trace_cores=[0], tmpdir="/tmp/ktrace")
    res = run_bass_kernel_spmd(nc, in_maps, list(range(NCORES)), **kw)
    if TRACE:
        print(f"HW exec time: {res.exec_time_ns} ns")

    out_mean = np.concatenate([res.results[i]["omT"].T for i in range(NCORES)], axis=0)
    out_sample = np.concatenate([res.results[i]["osT"].T for i in range(NCORES)], axis=0)
    out_logp = np.concatenate([res.results[i]["olT"].T for i in range(NCORES)], axis=0)
    return out_mean, out_sample, out_logp


# revision 33
# speedup vs baseline: 1.2109x; 1.2109x over previous
"""Trainium2 Bass kernel for the autoregressive policy head (nn_ADM_6511170421537).

v2: fp8 DoubleRow matmuls for the per-step layers.

Structure (per core, pure data parallelism over 8 cores):
  trunk:  h = relu(x@sW0+b) -> relu(@sW1+b) -> relu(@sW2+b)   [B,256] (bf16)
  steps i=0..7 (sequential in i, batch-parallel), fp8 e4m3 DoubleRow:
      x1 = relu((h8@(16*W_in[i][:256]) + means@(16*Wext) + 16*b_in[i])/16)
           - one DR matmul per batch tile (K=256 in one pass, 2x MACs)
           - means+bias via K=8 bf16 row-band matmuls (const-1 row at 32j+7)
           - evac: single DVE/ACT op (MULT 1/16, MAX 0) -> e4m3
      x2 = relu((x1@(16*W_h[i]))/16 + b_h[i])
           - DR matmuls; evac: ACT activation(Relu, bias, scale=1/16) -> bf16
  head (bf16, 4-way column-banded) + epilogue: as v1.

Accuracy: e4m3 on x1/x2 only (trunk + head bf16) simulates to ~1.0-1.5%
norm-rel vs the 2e-2 gate.

Evacuations are the binding resource (PSUM->SBUF runs at ~1 elem/cycle on
DVE/ACT, no perf modes for fp32 PSUM sources); a greedy build-time load
balancer assigns each evac to the lighter engine (x2 pinned to ACT for the
fused scale+bias).
"""

import os

os.environ.setdefault("MYCRO_LOCAL_CACHE", "1")

import numpy as np
from contextlib import ExitStack

import concourse.bass as bass
import concourse.bacc as bacc
import concourse.mybir as mybir
import concourse.tile as tile
from concourse.bass_utils import run_bass_kernel_spmd

# ---- problem constants (hardcoded; kernel.py must be self-contained) ----
B = 65536
IN_DIM = 64
HID = 256
D = 8
NCORES = 8
BC = B // NCORES          # 8192 rows per core
BT = 512                  # batch tile (one fp32 PSUM bank of free dim)
NT = BC // BT             # 16 tiles per core
GRP = 4                   # tiles per group
NG = NT // GRP            # 4 groups
WAVEG = 4                 # groups per wave (single wave, 4-deep interleave)
LOG_2PI = float(np.log(2.0 * np.pi))
WS = 16.0                 # fp8 weight prescale
INV = 1.0 / WS

F32 = mybir.dt.float32
BF16 = mybir.dt.bfloat16
FP8 = mybir.dt.float8e4
RELU = mybir.ActivationFunctionType.Relu
EXP = mybir.ActivationFunctionType.Exp
ADD = mybir.AluOpType.add
MAX = mybir.AluOpType.max
MIN = mybir.AluOpType.min
MULT = mybir.AluOpType.mult
DR = mybir.MatmulPerfMode.DoubleRow

TRACE = False           # test.py flips this to get the NTFF profile
_NC_CACHE = {}


def _build_bass():
    nc = bacc.Bacc()

    # x in "L0 row-tiling" layout: partition 64h+c holds feature c of tiles
    # j in {2h, 2h+1} of each group; free = (group, j%2, batch)
    xT = nc.declare_dram_parameter("xT", [128, BC // 2], BF16, isOutput=False)
    epsE = nc.declare_dram_parameter("epsE", [128, BT], BF16, isOutput=False)
    # wa: L0 weights (sW0 duplicated in both 64-row halves)
    wa = nc.declare_dram_parameter("wa", [128, HID], BF16, isOutput=False)
    # wt8: [p, tl(0=L1,1=L2), u, mf] = 16*sW[128u+p, mf], e4m3
    wt8 = nc.declare_dram_parameter("wt8", [128, 2 * 2 * HID], FP8, isOutput=False)
    # wtb: trunk bias bands, row 32j+7 = 16*sb{1,2}
    wtb = nc.declare_dram_parameter("wtb", [128, 2 * 2 * 128], BF16, isOutput=False)
    # w8: [p, i, l(0=W_in,1=W_h), u(k-group), mf] = 16*W[i][128u+p, mf], e4m3
    w8 = nc.declare_dram_parameter("w8", [128, D * 2 * 2 * HID], FP8, isOutput=False)
    # wx2: correction weights, band 32j rows r; l=0 (x1): r<i ->
    # 16*W_in[i][256+r], r=7 -> 16*b_in[i]; l=1 (x2): r=7 -> 16*b_h[i]
    wx2 = nc.declare_dram_parameter("wx2", [128, D * 2 * 2 * 128], BF16, isOutput=False)
    wo = nc.declare_dram_parameter("wo", [128, D * 2 * 2], BF16, isOutput=False)
    bb = nc.declare_dram_parameter("bb", [128, 30], F32, isOutput=False)
    # mg init pattern: zeros with 1.0 at the const-bias rows 32j+7
    mgi = nc.declare_dram_parameter("mgi", [128, BT], BF16, isOutput=False)
    omT = nc.declare_dram_parameter("omT", [D, BC], F32, isOutput=True)
    osT = nc.declare_dram_parameter("osT", [D, BC], F32, isOutput=True)
    olT = nc.declare_dram_parameter("olT", [D, BC], F32, isOutput=True)

    # build-time greedy engine balancer for PSUM evacuations
    eload = {"act": 0.0, "dve": 0.0}

    def _cost(fd):
        # constants fit from measured per-instr times (ACT 1078ns, DVE
        # 1173ns at FD=1024)
        return {"act": (fd + 270.0) / 1.2, "dve": (fd + 100.0) / 0.96}

    def pick(fd, allowed=("act", "dve")):
        cost = _cost(fd)
        e = min(allowed, key=lambda k: eload[k] + cost[k])
        eload[e] += cost[e]
        return e

    def pick2(fd):
        """Both engines, lighter one first — a unit's two evacs run in
        parallel on ACT and DVE so their latency stays under the PE cover."""
        cost = _cost(fd)
        first = min(("act", "dve"), key=lambda k: eload[k])
        other = "dve" if first == "act" else "act"
        eload[first] += cost[first]
        eload[other] += cost[other]
        return first, other

    with tile.TileContext(nc) as tc, ExitStack() as ctx:
        wp = ctx.enter_context(tc.tile_pool(name="w", bufs=1))
        hpool = ctx.enter_context(tc.tile_pool(name="h8", bufs=NG))
        mgpool = ctx.enter_context(tc.tile_pool(name="mg", bufs=NG))
        tpool = ctx.enter_context(tc.tile_pool(name="tr", bufs=4))
        x1pool = ctx.enter_context(tc.tile_pool(name="x1", bufs=4))
        x2pool = ctx.enter_context(tc.tile_pool(name="x2", bufs=4))
        smpool = ctx.enter_context(tc.tile_pool(name="sm", bufs=8))
        opool = ctx.enter_context(tc.tile_pool(name="out", bufs=1))
        pspair = ctx.enter_context(tc.tile_pool(name="pspair", bufs=4, space="PSUM"))

        # ---- staged loads over the 3 DMA queues; critical path first
        xts_s = wp.tile([128, NG, 2, BT], BF16)
        xv = xT[:].rearrange("p (g u b) -> p g u b", g=NG, u=2)
        wa_s = wp.tile([128, HID], BF16)
        wt8_s = wp.tile([128, 2, 2, HID], FP8)
        wtb_s = wp.tile([128, 2, 2, 128], BF16)
        bb_s = wp.tile([128, 30], F32)
        w8_s = wp.tile([128, D, 2, 2, HID], FP8)
        wx2_s = wp.tile([128, D, 2, 2, 128], BF16)
        wo_s = wp.tile([128, D, 2, 2], BF16)
        eps_s = wp.tile([128, BT], BF16)

        # ---- PE warmup: dummy matmuls while the input DMAs stream
        wu = wp.tile([128, BT], BF16)
        nc.gpsimd.memset(wu[:], 0.0)
        psw = pspair.tile([128, 2, BT], F32, tag="pspair", name="warm")
        for r in range(9):
            nc.tensor.matmul(psw[:, r % 2, :], wu[0:128, 0:128], wu[:],
                             start=True, stop=True)

        # sync queue: x for groups 0-1 halves, w0, trunk DR weights
        nc.sync.dma_start(xts_s[0:64, 0, :, :], xv[0:64, 0, :, :])
        nc.sync.dma_start(wa_s[:], wa[:])
        nc.sync.dma_start(xts_s[64:128, 0, :, :], xv[64:128, 0, :, :])
        nc.sync.dma_start(wt8_s[:], wt8[:].rearrange("p (l u m) -> p l u m", l=2, u=2))
        nc.sync.dma_start(wtb_s[:], wtb[:].rearrange("p (l m c) -> p l m c", l=2, m=2))
        # scalar queue: wave tiles, biases, step weights
        nc.scalar.dma_start(xts_s[:, 1, :, :], xv[:, 1, :, :])
        nc.scalar.dma_start(bb_s[:], bb[:])
        nc.scalar.dma_start(w8_s[:], w8[:].rearrange("p (i l u m) -> p i l u m",
                                                     i=D, l=2, u=2))
        # gpsimd queue: remaining inputs, mg init, corrections, heads, eps
        nc.gpsimd.dma_start(xts_s[:, 2, :, :], xv[:, 2, :, :])
        nc.gpsimd.dma_start(xts_s[:, 3, :, :], xv[:, 3, :, :])
        nc.gpsimd.dma_start(wx2_s[:], wx2[:].rearrange("p (i l m c) -> p i l m c",
                                                       i=D, l=2, m=2))
        nc.gpsimd.dma_start(wo_s[:], wo[:].rearrange("p (i k c) -> p i k c",
                                                     i=D, k=2))
        nc.gpsimd.dma_start(eps_s[:], epsE[:])

        b0_s = bb_s[:, 0:2]
        bh_s = bb_s[:, 6:22].rearrange("p (i m) -> p i m", i=D)
        bo_s = bb_s[:, 22:30]

        def evac_relu_bias(dst, src, bias, eng):
            """relu(src + bias)."""
            if eng == "act":
                nc.scalar.activation(dst, src, RELU, bias=bias)
            else:
                nc.vector.tensor_scalar(dst, src, bias, 0.0, ADD, MAX)

        def evac_relu_scale(dst, src, eng):
            """relu(src/16) (bias already accumulated in PSUM)."""
            if eng == "act":
                nc.scalar.activation(dst, src, RELU, scale=INV)
            else:
                nc.vector.tensor_scalar(dst, src, INV, 0.0, MULT, MAX)

        # Activation tiles are per-group: [128, u(2), j(4), BT], feature
        # 128u + p; psum pair tiles [128, s(2), BT] hold slots j = 2p+s.

        def emit_dr_trunk(g, tl, rhs, dst, mg, tag):
            """fp8 DR 256->256 trunk layer over one group; bias via the K=8
            band matmuls against the const-1 row of mg (means still zero)."""
            for m in range(2):
                pss = [pspair.tile([128, 2, BT], F32, tag="pspair",
                                   name=f"ps{tag}{m}{p}") for p in range(2)]
                wv = wt8_s[:, tl, :, bass.ts(m, 128)]
                for p in range(2):
                    for s in range(2):
                        nc.tensor.matmul(
                            pss[p][:, s, :], wv, rhs[:, :, 2 * p + s, :],
                            start=True, stop=False, perf_mode=DR,
                        )
                for j in range(GRP):
                    nc.tensor.matmul(
                        pss[j // 2][:, j % 2, :],
                        wtb_s[32 * j : 32 * j + 8, tl, m, :],
                        mg[32 * j : 32 * j + 8, :],
                        start=False, stop=True,
                        tile_position=(32 * j, 0),
                    )
                engs = pick2(1024)
                for p in range(2):
                    evac_relu_scale(dst[:, m, 2 * p : 2 * p + 2, :],
                                    pss[p][:], engs[p])

        def trunk_wave(groups, states):
            """Trunk for all groups, interleaved at layer granularity."""
            hp = {g: tpool.tile([128, 2, 4, BT], FP8, tag="hp", name=f"hp{g}")
                  for g in groups}
            for g in groups:
                # L0 is K=64 bf16: tile pairs (0,2) and (1,3) run concurrently
                # in the two 64-row halves of the PE array (w0 duplicated).
                for m in range(2):
                    pss = [pspair.tile([128, 2, BT], F32, tag="pspair",
                                       name=f"pst0g{g}{m}{p}") for p in range(2)]
                    for j in (0, 2, 1, 3):
                        h = j // 2
                        nc.tensor.matmul(
                            pss[j // 2][:, j % 2, :],
                            wa_s[64 * h : 64 * h + IN_DIM, bass.ts(m, 128)],
                            xts_s[64 * h : 64 * h + IN_DIM, g, j % 2, :],
                            start=True, stop=True,
                            tile_position=(64 * h, 0),
                        )
                    engs = pick2(1024)
                    for p in range(2):
                        evac_relu_bias(hp[g][:, m, 2 * p : 2 * p + 2, :],
                                       pss[p][:], b0_s[:, m : m + 1], engs[p])
            hq = {g: tpool.tile([128, 2, 4, BT], FP8, tag="hq", name=f"hq{g}")
                  for g in groups}
            for g in groups:
                emit_dr_trunk(g, 0, hp[g], hq[g], states[g]["mg"], f"t1g{g}")
            for g in groups:
                # L2 evac writes the e4m3 h8 tile (consumed by the x1 DRs)
                emit_dr_trunk(g, 1, hq[g], states[g]["h"], states[g]["mg"],
                              f"t2g{g}")

        def step_x1(i, g, h8, mg):
            """x1 = relu((h@16W + means@16Wext + 16b)/16) via DR + K=8 bands."""
            x1t = x1pool.tile([128, 2, 4, BT], FP8, tag="x1", name=f"x1{g}")
            for m in range(2):
                pss = [pspair.tile([128, 2, BT], F32, tag="pspair",
                                   name=f"psl{i}g{g}{m}{p}") for p in range(2)]
                wv = w8_s[:, i, 0, :, bass.ts(m, 128)]
                for p in range(2):
                    for s in range(2):
                        nc.tensor.matmul(
                            pss[p][:, s, :], wv, h8[:, :, 2 * p + s, :],
                            start=True, stop=False, perf_mode=DR,
                        )
                for j in range(GRP):
                    nc.tensor.matmul(
                        pss[j // 2][:, j % 2, :],
                        wx2_s[32 * j : 32 * j + 8, i, 0, m, :],
                        mg[32 * j : 32 * j + 8, :],
                        start=False, stop=True,
                        tile_position=(32 * j, 0),
                    )
                engs = pick2(1024)
                for p in range(2):
                    evac_relu_scale(x1t[:, m, 2 * p : 2 * p + 2, :],
                                    pss[p][:], engs[p])
            return x1t

        def step_x2(i, g, x1t, mg):
            """x2 = relu((x1@16W + 16b)/16) via DR + K=8 bias band (bf16)."""
            x2t = x2pool.tile([128, 2, 4, BT], BF16, tag="x2", name=f"x2{g}")
            for m in range(2):
                pss = [pspair.tile([128, 2, BT], F32, tag="pspair",
                                   name=f"psh{i}g{g}{m}{p}") for p in range(2)]
                wv = w8_s[:, i, 1, :, bass.ts(m, 128)]
                for p in range(2):
                    for s in range(2):
                        nc.tensor.matmul(
                            pss[p][:, s, :], wv, x1t[:, :, 2 * p + s, :],
                            start=True, stop=False, perf_mode=DR,
                        )
                for j in range(GRP):
                    nc.tensor.matmul(
                        pss[j // 2][:, j % 2, :],
                        wx2_s[32 * j : 32 * j + 8, i, 1, m, :],
                        mg[32 * j : 32 * j + 8, :],
                        start=False, stop=True,
                        tile_position=(32 * j, 0),
                    )
                engs = pick2(1024)
                for p in range(2):
                    evac_relu_scale(x2t[:, m, 2 * p : 2 * p + 2, :],
                                    pss[p][:], engs[p])
            return x2t

        def step_head(i, g, x2t, mg, epi):
            """head: 4 tiles' M=2 matmuls concurrent in PE column groups."""
            pst = pspair.tile([128, 2, BT], F32, tag="pspair", name=f"pshd{i}g{g}")
            pso = pst[:, 0, :]
            for k in range(2):
                for j in range(GRP):
                    nc.tensor.matmul(
                        pso[32 * j : 32 * j + 2, :],
                        wo_s[:, i, k, :],
                        x2t[:, k, j, :],
                        start=(k == 0), stop=(k == 1),
                        tile_position=(0, 32 * j),
                    )
            sm = smpool.tile([128, BT], BF16, tag="sm", name=f"sm{g}_{i}")
            evac_relu_bias(sm[0:98, :], pso[0:98, :], bo_s[0:98, i : i + 1],
                           pick(512))
            # scatter: mean_j -> band row 32j+i of mg, ls_j -> 32j+8+i
            # (or for the last step straight into the epilogue tile).
            smv = sm[:].rearrange("(j r) b -> j r b", j=4)
            mgv = mg[:].rearrange("(j q r) b -> j q r b", j=4, q=4)
            if i < D - 1:
                dst_m, dst_l = mgv[:, 0, i, :], mgv[:, 1, i, :]
            else:
                # epilogue layout: partition 8t + i
                dst_m = epi[32 * g + 7 : 32 * g + 32 : 8, 0, :]
                dst_l = epi[32 * g + 7 : 32 * g + 32 : 8, 1, :]
            nc.sync.dma_start(dst_m, smv[:, 0, :])
            nc.gpsimd.dma_start(dst_l, smv[:, 1, :])

        def emit_gathers(g, mg, epi):
            """Move steps 0..6 means/log_stds of group g into the epilogue
            layout (partition 8t + i)."""
            for j in range(GRP):
                t = GRP * g + j
                nc.gpsimd.dma_start(epi[8 * t : 8 * t + 7, 0, :],
                                    mg[32 * j : 32 * j + 7, :])
                nc.gpsimd.dma_start(epi[8 * t : 8 * t + 7, 1, :],
                                    mg[32 * j + 8 : 32 * j + 15, :])

        state = {}
        for g in range(NG):
            state[g] = dict(
                h=hpool.tile([128, 2, 4, BT], FP8, tag="h8", name=f"h8_{g}"),
                mg=mgpool.tile([128, BT], BF16, tag="mg", name=f"mg{g}"),
            )
        # correction bands: zero the mean rows (the K=8 stationary reads rows
        # i..6 before they're written) and set the const-1 bias row 32j+7;
        # the trunk's bias bands also read these, so load them first (sync q)
        for g in range(NG):
            nc.sync.dma_start(state[g]["mg"][:], mgi[:])

        # epilogue tile: [p = 8*tile + step, (mean, log_std), batch]
        epi = opool.tile([128, 2, BT], BF16, tag="epi")
        lsc = opool.tile([128, BT], BF16, tag="lsc")
        mean_f = opool.tile([128, BT], F32, tag="mean_f")
        st = opool.tile([128, BT], BF16, tag="st")
        se = opool.tile([128, BT], BF16, tag="se")
        smp = opool.tile([128, BT], F32, tag="smp")
        sq2 = opool.tile([128, BT], BF16, tag="sq2")
        lp = opool.tile([128, BT], F32, tag="lp")

        # -0.5*eps^2 - 0.5*log(2pi): precomputed while the trunk runs
        nc.gpsimd.tensor_mul(sq2[:], eps_s[:], eps_s[:])
        nc.gpsimd.tensor_scalar(sq2[:], sq2[:], -0.5, -0.5 * LOG_2PI, MULT, ADD)

        omV = omT[:].rearrange("i (t b) -> t i b", t=NT)
        osV = osT[:].rearrange("i (t b) -> t i b", t=NT)
        olV = olT[:].rearrange("i (t b) -> t i b", t=NT)

        def emit_epi_half(hf):
            """Epilogue for partitions [64*hf, 64*hf+64) (tiles 8*hf..)."""
            sub = slice(64 * hf, 64 * hf + 64)
            tl = slice(8 * hf, 8 * hf + 8)
            nc.vector.tensor_single_scalar(lsc[sub, :], epi[sub, 1, :], 2.0, MIN)
            nc.vector.tensor_copy(mean_f[sub, :], epi[sub, 0, :])
            nc.sync.dma_start(omV[tl], mean_f[sub, :])
            nc.scalar.activation(st[sub, :], lsc[sub, :], EXP)
            nc.vector.tensor_mul(se[sub, :], st[sub, :], eps_s[sub, :])
            nc.vector.tensor_add(smp[sub, :], se[sub, :], mean_f[sub, :])
            nc.scalar.dma_start(osV[tl], smp[sub, :])
            nc.vector.tensor_sub(lp[sub, :], sq2[sub, :], lsc[sub, :])
            nc.gpsimd.dma_start(olV[tl], lp[sub, :])

        groups = list(range(NG))
        trunk_wave(groups, state)
        for i in range(D):
            x1s = {g: step_x1(i, g, state[g]["h"], state[g]["mg"])
                   for g in groups}
            if i == D - 1:
                for g in groups:
                    emit_gathers(g, state[g]["mg"], epi)
            x2s = {g: step_x2(i, g, x1s[g], state[g]["mg"]) for g in groups}
            for g in groups:
                step_head(i, g, x2s[g], state[g]["mg"], epi)
                # tiles 0-7 (groups 0,1) complete first: overlap their
                # epilogue under the last groups' heads
                if i == D - 1 and g == 1:
                    emit_epi_half(0)
        emit_epi_half(1)

    nc.compile()
    return nc


def _get_nc():
    if "nc" not in _NC_CACHE:
        _NC_CACHE["nc"] = _build_bass()
    return _NC_CACHE["nc"]


def kernel(**inputs):
    import ml_dtypes

    bf16 = ml_dtypes.bfloat16
    e4m3 = ml_dtypes.float8_e4m3
    inp = {k: np.ascontiguousarray(np.asarray(v, dtype=np.float32))
           for k, v in inputs.items()}
    x = inp["inputs"]
    eps = inp["eps"]
    W_in, b_in = inp["W_in"], inp["b_in"]
    W_h, b_h = inp["W_h"], inp["b_h"]
    W_out, b_out = inp["W_out"], inp["b_out"]

    def cb(a):
        return np.ascontiguousarray(a.astype(bf16))

    def c8(a):
        return np.ascontiguousarray(np.clip(a, -240.0, 240.0).astype(e4m3))

    c = np.ascontiguousarray

    # w8: [p, i, l, u, mf] = 16*W[i][128u+p, mf]
    wi = (W_in[:, :HID, :] * WS).reshape(D, 2, 128, HID).transpose(2, 0, 1, 3)
    wh = (W_h * WS).reshape(D, 2, 128, HID).transpose(2, 0, 1, 3)
    w8_np = np.stack([wi, wh], axis=2)  # [128, D, l, u, HID]

    # wx2 correction bands: [32j + r, i, l, m, c]; l=0: r<i ->
    # 16*W_in[i][256+r], r=7 -> 16*b_in[i]; l=1: r=7 -> 16*b_h[i]
    base = np.zeros((32, D, 2, 2, 128), np.float32)
    for i in range(D):
        if i > 0:
            base[:i, i, 0] = (W_in[i, HID : HID + i, :] * WS).reshape(i, 2, 128)
        base[7, i, 0] = (b_in[i] * WS).reshape(2, 128)
        base[7, i, 1] = (b_h[i] * WS).reshape(2, 128)
    wx2_np = np.tile(base, (4, 1, 1, 1, 1))

    bo_band = np.zeros((128, D), np.float32)
    for j in range(4):
        for ch in range(2):
            bo_band[32 * j + ch, :] = b_out[:, ch]

    wa_np = np.zeros((128, HID), np.float32)
    wa_np[:IN_DIM] = inp["sW0"]
    wa_np[64 : 64 + IN_DIM] = inp["sW0"]  # L0 row-tiling duplicate

    # trunk DR weights (x16, e4m3) and bias bands
    wt8_np = np.stack([
        (inp["sW1"] * WS).reshape(2, 128, HID).transpose(1, 0, 2),
        (inp["sW2"] * WS).reshape(2, 128, HID).transpose(1, 0, 2),
    ], axis=1)  # [128, tl, u, HID]
    wtb_np = np.zeros((32, 2, 2, 128), np.float32)
    wtb_np[7, 0] = (inp["sb1"] * WS).reshape(2, 128)
    wtb_np[7, 1] = (inp["sb2"] * WS).reshape(2, 128)
    wtb_np = np.tile(wtb_np, (4, 1, 1, 1))

    wo_np = W_out.reshape(D, 2, 128, 2).transpose(2, 0, 1, 3)

    bb_np = np.concatenate([
        inp["sb0"].reshape(2, 128).T, inp["sb1"].reshape(2, 128).T,
        inp["sb2"].reshape(2, 128).T,
        b_h.reshape(D, 2, 128).transpose(2, 0, 1).reshape(128, -1),
        bo_band,
    ], axis=1)

    mgi_np = np.zeros((128, BT), np.float32)
    for j in range(4):
        mgi_np[32 * j + 7, :] = 1.0

    shared = {
        "wa": cb(wa_np),
        "wt8": c8(wt8_np).reshape(128, -1),
        "wtb": cb(wtb_np.reshape(128, -1)),
        "w8": c8(w8_np).reshape(128, -1),
        "wx2": cb(wx2_np.reshape(128, -1)),
        "wo": cb(wo_np.reshape(128, -1)),
        "bb": c(bb_np),
        "mgi": cb(mgi_np),
    }

    in_maps = []
    for core in range(NCORES):
        sl = slice(core * BC, (core + 1) * BC)
        m = dict(shared)
        xg = x[sl].T.reshape(IN_DIM, 4, 2, 2, BT).transpose(2, 0, 1, 3, 4)
        m["xT"] = cb(xg.reshape(128, BC // 2))
        m["epsE"] = cb(eps[sl].T.reshape(D, NT, BT).transpose(1, 0, 2).reshape(128, BT))
        in_maps.append(m)

    nc = _get_nc()
    kw = {}
    if TRACE:
        import shutil

        shutil.rmtree("/tmp/ktrace", ignore_errors=True)
        os.makedirs("/tmp/ktrace", exist_ok=True)
        kw = dict(trace=True, trace_cores=[0], tmpdir="/tmp/ktrace")
    res = run_bass_kernel_spmd(nc, in_maps, list(range(NCORES)), **kw)
    if TRACE:
        print(f"HW exec time: {res.exec_time_ns} ns")

    out_mean = np.concatenate([res.results[i]["omT"].T for i in range(NCORES)], axis=0)
    out_sample = np.concatenate([res.results[i]["osT"].T for i in range(NCORES)], axis=0)
    out_logp = np.concatenate([res.results[i]["olT"].T for i in range(NCORES)], axis=0)
    return out_mean, out_sample, out_logp
